# revision 3
# baseline (speedup 1.0000x reference)
"""Trainium2 Bass kernel for nn_MultiHeadAttention_17386027615012.

MHA variant: softmax over the HEAD axis (dim=1), 1/sqrt(emb) applied AFTER
softmax (folded into proj_w host-side). Softmax-over-heads makes every (q, k)
independent -> shard flattened (batch, seq) query rows over 8 cores
(b = core // 4, 1024-row q-chunk = core % 4), zero inter-core comms; each
core computes its batch's full K/V (4x redundant).

v2: fused single-pass streaming, everything SBUF-resident (no DRAM staging).
Per 512-col k-block: project K^T/V from streamed x^T chunks, then per 128-k
chunk: energy matmuls (bf16) -> exp (ACT) -> head-sum (PE identity-matmul for
heads 0-3 + DVE bf16 tree for 4-7) -> fast reciprocal -> in-place normalize
(DVE+Pool split) -> att@V accumulated in PSUM per 4-kc segment. PSUM->SBUF
staging copies run on GpSimd (Pool) to keep ACT exclusively on exp.
"""
import sys

sys.path.insert(0, "/opt/trn_rl_repo")

import numpy as np
import ml_dtypes
from contextlib import ExitStack

import concourse.bass as bass
import concourse.tile as tile
from concourse import bacc, mybir
from concourse import bass_utils

F32 = mybir.dt.float32
BF16 = mybir.dt.bfloat16
EXPF = mybir.ActivationFunctionType.Exp
IDENT = mybir.ActivationFunctionType.Identity

B, N, E, H, D = 2, 4096, 768, 8, 96
NCORES = 4 * B
QC = N // 4          # 1024 q rows per core
KB = 512             # k-block (projection granularity)
NKB = N // KB        # 8
SEG = 4              # k-chunks (of 128) per Ot psum accumulation segment
NKC = N // 128       # 32
NE = E // 128        # 6

N_PE_SUM = 4         # heads summed via PE identity matmul (rest on DVE)
N_POOL_MUL = 3       # normalize muls routed to Pool engine (rest on DVE)


def build(use_bias: bool):
    nc = bacc.Bacc("TRN2", debug=False)
    xt = nc.dram_tensor("xt", (E, N), BF16, kind="ExternalInput").ap()
    xtq = nc.dram_tensor("xtq", (E, QC), BF16, kind="ExternalInput").ap()
    wq = nc.dram_tensor("wq", (E, E), BF16, kind="ExternalInput").ap()
    wk = nc.dram_tensor("wk", (E, E), BF16, kind="ExternalInput").ap()
    wv = nc.dram_tensor("wv", (E, E), BF16, kind="ExternalInput").ap()
    pw = nc.dram_tensor("pw", (E, E), BF16, kind="ExternalInput").ap()
    bqk = nc.dram_tensor("bqk", (2, H, D), F32, kind="ExternalInput").ap()
    bv = nc.dram_tensor("bv", (1, E), BF16, kind="ExternalInput").ap()
    ident = nc.dram_tensor("ident", (128, 128), BF16, kind="ExternalInput").ap()
    out = nc.dram_tensor("out", (QC, E), F32, kind="ExternalOutput").ap()

    with tile.TileContext(nc) as tc, ExitStack() as ctx:
        # ---- persistent pools (whole kernel) ----
        wpool = ctx.enter_context(tc.tile_pool(name="wpool", bufs=1))
        wks, wvs = [], []
        for e in range(NE):
            wk_t = wpool.tile([128, E], BF16, name=f"wk{e}")
            nc.sync.dma_start(wk_t[:], wk[e * 128:(e + 1) * 128, :])
            wks.append(wk_t)
        for e in range(NE):
            wv_t = wpool.tile([128, E], BF16, name=f"wv{e}")
            nc.sync.dma_start(wv_t[:], wv[e * 128:(e + 1) * 128, :])
            wvs.append(wv_t)
        id_t = wpool.tile([128, 128], BF16, name="id_t")
        nc.sync.dma_start(id_t[:], ident[:, :])
        qsl = wpool.tile([D, H * QC], BF16, name="qsl")
        ot_sb = []
        for h in range(H):
            o_t = wpool.tile([D, QC], F32, name=f"ot{h}")
            ot_sb.append(o_t)
        if use_bias:
            bqk_t = wpool.tile([D, 2 * H], F32, name="bqk_t")
            nc.sync.dma_start(
                bqk_t.rearrange("d (c h) -> d c h", c=2),
                bqk.rearrange("c h d -> d c h"),
            )
            ones_t = wpool.tile([1, 128], BF16, name="ones_t")
            nc.vector.memset(ones_t[:], 1.0)
            bv_t = wpool.tile([1, E], BF16, name="bv_t")
            nc.sync.dma_start(bv_t[:], bv[:, :])

        # ---- phase 0: Q projection ----
        with ExitStack() as actx:
            qppool = actx.enter_context(tc.tile_pool(name="qppool", bufs=1))
            with tc.tile_pool(name="qpsum", bufs=2, space="PSUM") as qpsum:
                xqs = []
                for e in range(NE):
                    xq_t = qppool.tile([128, QC], BF16, name=f"xq{e}")
                    nc.sync.dma_start(xq_t[:], xtq[e * 128:(e + 1) * 128, :])
                    xqs.append(xq_t)
                wqs = []
                for e in range(NE):
                    wq_t = qppool.tile([128, E], BF16, name=f"wq{e}")
                    nc.sync.dma_start(wq_t[:], wq[e * 128:(e + 1) * 128, :])
                    wqs.append(wq_t)
                for h in range(H):
                    qp = qpsum.tile([128, QC], F32, name="qp")
                    for i in range(2):
                        for e in range(NE):
                            nc.tensor.matmul(
                                qp[0:D, i * 512:(i + 1) * 512],
                                wqs[e][:, h * D:(h + 1) * D],
                                xqs[e][:, i * 512:(i + 1) * 512],
                                start=(e == 0), stop=(e == NE - 1),
                            )
                    if use_bias:
                        nc.scalar.activation(
                            qsl[:, h * QC:(h + 1) * QC], qp[0:D, :],
                            IDENT, bias=bqk_t[:, h:h + 1],
                        )
                    else:
                        nc.scalar.copy(
                            qsl[:, h * QC:(h + 1) * QC], qp[0:D, :])

        # ---- main fused loop over k-blocks ----
        with ExitStack() as bctx:
            xpool = bctx.enter_context(tc.tile_pool(name="xpool", bufs=1))
            ktpool = bctx.enter_context(tc.tile_pool(name="ktpool", bufs=2))
            vtpool = bctx.enter_context(tc.tile_pool(name="vtpool", bufs=1))
            expool = bctx.enter_context(tc.tile_pool(name="expool", bufs=1))
            spool = bctx.enter_context(tc.tile_pool(name="spool", bufs=2))
            epsum = bctx.enter_context(
                tc.tile_pool(name="epsum", bufs=2, space="PSUM"))
            mpsum = bctx.enter_context(
                tc.tile_pool(name="mpsum", bufs=2, space="PSUM"))

            for kb in range(NKB):
                # stream x^T chunk for this k-block
                xks = []
                for e in range(NE):
                    xk_t = xpool.tile([128, KB], BF16, name="xk",
                                      tag="xk", bufs=NE + 2)
                    nc.sync.dma_start(
                        xk_t[:],
                        xt[e * 128:(e + 1) * 128, kb * KB:(kb + 1) * KB])
                    xks.append(xk_t)
                # K^T projection -> kt [d, h*KB]
                ktt = ktpool.tile([D, H * KB], BF16, name="ktt", tag="ktt",
                                  bufs=2)
                for h in range(H):
                    mp = mpsum.tile([128, QC], F32, name="mp", tag="mp")
                    for e in range(NE):
                        nc.tensor.matmul(
                            mp[0:D, 0:KB],
                            wks[e][:, h * D:(h + 1) * D],
                            xks[e][:],
                            start=(e == 0), stop=(e == NE - 1),
                        )
                    if use_bias:
                        nc.scalar.activation(
                            ktt[:, h * KB:(h + 1) * KB], mp[0:D, 0:KB],
                            IDENT, bias=bqk_t[:, H + h:H + h + 1],
                        )
                    else:
                        nc.scalar.copy(
                            ktt[:, h * KB:(h + 1) * KB], mp[0:D, 0:KB])
                # V projection -> vt [k, (h d)] per 128-k chunk
                vts = []
                for kc4 in range(SEG):
                    mp = mpsum.tile([128, QC], F32, name="mp", tag="mp")
                    for e in range(NE):
                        nc.tensor.matmul(
                            mp[:, 0:512],
                            xks[e][:, kc4 * 128:(kc4 + 1) * 128],
                            wvs[e][:, 0:512],
                            start=(e == 0), stop=(e == NE - 1),
                        )
                        nc.tensor.matmul(
                            mp[:, 512:E],
                            xks[e][:, kc4 * 128:(kc4 + 1) * 128],
                            wvs[e][:, 512:E],
                            start=(e == 0), stop=(e == NE - 1),
                        )
                    if use_bias:
                        nc.tensor.matmul(
                            mp[:, 0:512], ones_t[:, 0:128], bv_t[:, 0:512],
                            start=False, stop=True, skip_group_check=True,
                        )
                        nc.tensor.matmul(
                            mp[:, 512:E], ones_t[:, 0:128], bv_t[:, 512:E],
                            start=False, stop=True, skip_group_check=True,
                        )
                    v_t = vtpool.tile([128, E], BF16, name="vt", tag="vt",
                                      bufs=SEG + 2)
                    nc.scalar.copy(v_t[:], mp[:, 0:E])
                    vts.append(v_t)

                # attention over the 4 k-chunks of this block
                exalls = []
                for kc4 in range(SEG):
                    exa = expool.tile([128, H * QC], BF16, name="exa",
                                      tag="exa", bufs=SEG + 1)
                    for h in range(H):
                        ep = epsum.tile([128, QC], F32, name="ep", tag="ep")
                        for i in range(2):
                            nc.tensor.matmul(
                                ep[:, i * 512:(i + 1) * 512],
                                ktt[:, h * KB + kc4 * 128:
                                    h * KB + (kc4 + 1) * 128],
                                qsl[:, h * QC + i * 512:
                                    h * QC + (i + 1) * 512],
                                start=True, stop=True,
                            )
                        nc.scalar.activation(
                            exa[:, h * QC:(h + 1) * QC], ep[:], EXPF)
                    # head-sum: heads 0..N_PE_SUM-1 on PE, rest DVE tree
                    sp = mpsum.tile([128, QC], F32, name="mp", tag="mp")
                    for i in range(2):
                        for h in range(N_PE_SUM):
                            nc.tensor.matmul(
                                sp[:, i * 512:(i + 1) * 512],
                                id_t[:],
                                exa[:, h * QC + i * 512:h * QC + (i + 1) * 512],
                                start=(h == 0), stop=(h == N_PE_SUM - 1),
                            )
                    sA = spool.tile([128, QC], BF16, name="sA", tag="sA")
                    sB = spool.tile([128, QC], BF16, name="sB", tag="sB")
                    nc.vector.tensor_add(
                        sA[:], exa[:, 4 * QC:5 * QC], exa[:, 5 * QC:6 * QC])
                    nc.vector.tensor_add(
                        sB[:], exa[:, 6 * QC:7 * QC], exa[:, 7 * QC:8 * QC])
                    nc.vector.tensor_add(sA[:], sA[:], sB[:])
                    sful = spool.tile([128, QC], F32, name="sful", tag="sful",
                                      bufs=1)
                    nc.vector.tensor_add(sful[:], sp[:], sA[:])
                    r32 = spool.tile([128, QC], F32, name="r32", tag="r32")
                    nc.vector.reciprocal_approx_fast(r32[:], sful[:])
                    r16 = spool.tile([128, QC], BF16, name="r16", tag="r16")
                    nc.vector.tensor_scalar_min(r16[:], r32[:], 3e38)
                    # normalize in place: att_h = ex_h * r
                    for h in range(H):
                        eng = nc.gpsimd if h < N_POOL_MUL else nc.vector
                        eng.tensor_mul(
                            exa[:, h * QC:(h + 1) * QC],
                            exa[:, h * QC:(h + 1) * QC],
                            r16[:],
                        )
                    exalls.append(exa)

                # Ot for this block (= one 4-kc psum segment)
                for h in range(H):
                    op = mpsum.tile([128, QC], F32, name="mp", tag="mp")
                    for kc4 in range(SEG):
                        for i in range(2):
                            nc.tensor.matmul(
                                op[0:D, i * 512:(i + 1) * 512],
                                vts[kc4][:, h * D:(h + 1) * D],
                                exalls[kc4][:, h * QC + i * 512:
                                             h * QC + (i + 1) * 512],
                                start=(kc4 == 0), stop=(kc4 == SEG - 1),
                            )
                    if kb == 0:
                        nc.scalar.copy(ot_sb[h][:], op[0:D, :])
                    else:
                        nc.vector.tensor_add(
                            ot_sb[h][:], ot_sb[h][:], op[0:D, :])

        # ---- output projection ----
        with ExitStack() as cctx:
            ppool = cctx.enter_context(tc.tile_pool(name="ppool", bufs=1))
            ostp = cctx.enter_context(tc.tile_pool(name="ostp", bufs=2))
            pws = []
            for h in range(H):
                pw_t = ppool.tile([D, E], BF16, name=f"pw{h}")
                nc.sync.dma_start(pw_t[:], pw[h * D:(h + 1) * D, :])
                pws.append(pw_t)
            otr = []
            for h in range(H):
                otr_t = ppool.tile([D, QC], BF16, name=f"otr{h}")
                nc.gpsimd.tensor_copy(otr_t[:], ot_sb[h][:])
                otr.append(otr_t)
            with tc.tile_pool(name="pop", bufs=2, space="PSUM") as pop:
                for qb in range(QC // 128):
                    po = pop.tile([128, E], F32, name="po")
                    for h in range(H):
                        lhs = otr[h][:, qb * 128:(qb + 1) * 128]
                        nc.tensor.matmul(
                            po[:, 0:512], lhs, pws[h][:, 0:512],
                            start=(h == 0), stop=(h == H - 1))
                        nc.tensor.matmul(
                            po[:, 512:E], lhs, pws[h][:, 512:E],
                            start=(h == 0), stop=(h == H - 1))
                    ost = ostp.tile([128, E], F32, name="ost")
                    nc.scalar.copy(ost[:], po[:])
                    nc.sync.dma_start(out[qb * 128:(qb + 1) * 128, :], ost[:])

    nc.compile()
    return nc


_CACHE = {}


def _get_program(use_bias: bool):
    if use_bias not in _CACHE:
        _CACHE[use_bias] = build(use_bias)
    return _CACHE[use_bias]


def _prep_inputs(x, qkv_w, qkv_b, proj_w):
    bf = ml_dtypes.bfloat16
    qw = np.ascontiguousarray(qkv_w.reshape(E, H, D, 3))
    wq = np.ascontiguousarray(qw[..., 0].reshape(E, E)).astype(bf)
    wk = np.ascontiguousarray(qw[..., 1].reshape(E, E)).astype(bf)
    wv = np.ascontiguousarray(qw[..., 2].reshape(E, E)).astype(bf)
    pw = np.ascontiguousarray(
        proj_w / np.sqrt(np.float32(E))).astype(bf)
    qb = qkv_b.reshape(H, D, 3)
    bqk = np.ascontiguousarray(
        np.stack([qb[..., 0], qb[..., 1]], axis=0)).astype(np.float32)
    bv = np.ascontiguousarray(qb[..., 2].reshape(1, E)).astype(bf)
    xts = [np.ascontiguousarray(x[b].T).astype(bf) for b in range(B)]
    in_maps = []
    for c in range(NCORES):
        b, qi = c // 4, c % 4
        in_maps.append({
            "xt": xts[b],
            "xtq": np.ascontiguousarray(xts[b][:, qi * QC:(qi + 1) * QC]),
            "wq": wq, "wk": wk, "wv": wv, "pw": pw,
            "bqk": bqk, "bv": bv,
            "ident": np.eye(128, dtype=bf),
        })
    return in_maps


def kernel(x, qkv_w, qkv_b, proj_w, proj_b, _trace=False):
    x = np.asarray(x, dtype=np.float32)
    qkv_w = np.asarray(qkv_w, dtype=np.float32)
    qkv_b = np.asarray(qkv_b, dtype=np.float32)
    proj_w = np.asarray(proj_w, dtype=np.float32)
    proj_b = np.asarray(proj_b, dtype=np.float32)

    use_bias = bool(np.any(qkv_b))
    nc = _get_program(use_bias)
    in_maps = _prep_inputs(x, qkv_w, qkv_b, proj_w)
    res = bass_utils.run_bass_kernel_spmd(
        nc, in_maps, core_ids=list(range(NCORES)), trace=_trace)
    outf = np.empty((B, N, E), dtype=np.float32)
    for c in range(NCORES):
        b, qi = c // 4, c % 4
        outf[b, qi * QC:(qi + 1) * QC, :] = res.results[c]["out"]
    if np.any(proj_b):
        outf += proj_b[None, None, :]
    if _trace:
        kernel.last_exec_time_ns = res.exec_time_ns
        kernel.last_results = res
    return outf


# revision 8
# speedup vs baseline: 1.2808x; 1.2808x over previous
"""Trainium2 Bass kernel for nn_MultiHeadAttention_17386027615012.

MHA variant: softmax over the HEAD axis (dim=1), 1/sqrt(emb) applied AFTER
softmax (folded into proj_w host-side). Softmax-over-heads makes every (q, k)
independent -> shard flattened (batch, seq) query rows over 8 cores
(b = core // 4, 1024-row q-chunk = core % 4), zero inter-core comms; each
core computes its batch's full K/V (4x redundant).

v2.1: fused single-pass streaming, SBUF-resident (no DRAM staging).
All PSUM work shares one pool of two [128,2048] slots (8 banks):
 - K proj packs 4 heads/slot, V proj 2 k-chunks/slot, energy 2 heads/slot
   (-> one wide [128,2048] exp ACTIVATE per head-pair), Ot 2 heads/slot.
 - Ot accumulation across 4-kc segments seeds each new PSUM group with the
   running total via an identity matmul (fp16), so cross-segment adds cost
   PE columns instead of DVE passes; drains are wide ACT copies into a
   single fp16 ot_sb tile that the output projection consumes directly.
Softmax: wide DVE bf16 tree sum (4096/2048/1024), fast reciprocal, clamp on
GpSimd, out-of-place bf16 normalize muls on DVE (2x mode).
K/V projections for block kb+1 issue before Ot(kb) to hide softmax latency.
"""
import sys

sys.path.insert(0, "/opt/trn_rl_repo")

import numpy as np
import ml_dtypes
from contextlib import ExitStack

import concourse.bass as bass
import concourse.tile as tile
from concourse import bacc, mybir
from concourse import bass_utils

F32 = mybir.dt.float32
BF16 = mybir.dt.bfloat16
FP16 = mybir.dt.float16
EXPF = mybir.ActivationFunctionType.Exp
IDENT = mybir.ActivationFunctionType.Identity

B, N, E, H, D = 2, 4096, 768, 8, 96
NCORES = 4 * B
QC = N // 4          # 1024 q rows per core
KB = 512             # k-block (projection granularity)
NKB = N // KB        # 8
SEG = 4              # k-chunks per Ot psum segment (= one k-block)
NE = E // 128        # 6


DEBUG = False


def build(use_bias: bool):
    nc = bacc.Bacc("TRN2", debug=False)
    xt = nc.dram_tensor("xt", (E, N), BF16, kind="ExternalInput").ap()
    xtq = nc.dram_tensor("xtq", (E, QC), BF16, kind="ExternalInput").ap()
    wq = nc.dram_tensor("wq", (E, E), BF16, kind="ExternalInput").ap()
    wk = nc.dram_tensor("wk", (E, E), BF16, kind="ExternalInput").ap()
    wv = nc.dram_tensor("wv", (E, E), BF16, kind="ExternalInput").ap()
    pw = nc.dram_tensor("pw", (E, E), FP16, kind="ExternalInput").ap()
    bqk = nc.dram_tensor("bqk", (2, H, D), F32, kind="ExternalInput").ap()
    bv = nc.dram_tensor("bv", (1, E), BF16, kind="ExternalInput").ap()
    ident = nc.dram_tensor("ident", (128, 128), FP16, kind="ExternalInput").ap()
    out = nc.dram_tensor("out", (QC, E), F32, kind="ExternalOutput").ap()
    if DEBUG:
        dbg_qsl = nc.dram_tensor("dbg_qsl", (D, H * QC), BF16,
                                 kind="ExternalOutput").ap()
        dbg_ktt = nc.dram_tensor("dbg_ktt", (D, H * KB), BF16,
                                 kind="ExternalOutput").ap()
        dbg_vt = nc.dram_tensor("dbg_vt", (128, 2 * E), BF16,
                                kind="ExternalOutput").ap()
        dbg_exa = nc.dram_tensor("dbg_exa", (128, H * QC), BF16,
                                 kind="ExternalOutput").ap()
        dbg_att = nc.dram_tensor("dbg_att", (128, H * QC), BF16,
                                 kind="ExternalOutput").ap()
        dbg_ot = nc.dram_tensor("dbg_ot", (D, H * QC), FP16,
                                kind="ExternalOutput").ap()

    with tile.TileContext(nc) as tc, ExitStack() as ctx:
        # ---- persistent pools ----
        wpool = ctx.enter_context(tc.tile_pool(name="wpool", bufs=1))
        wks, wvs = [], []
        for e in range(NE):
            wk_t = wpool.tile([128, E], BF16, name=f"wk{e}")
            nc.sync.dma_start(wk_t[:], wk[e * 128:(e + 1) * 128, :])
            wks.append(wk_t)
        for e in range(NE):
            wv_t = wpool.tile([128, E], BF16, name=f"wv{e}")
            nc.sync.dma_start(wv_t[:], wv[e * 128:(e + 1) * 128, :])
            wvs.append(wv_t)
        id_t = wpool.tile([128, 128], FP16, name="id_t")
        nc.sync.dma_start(id_t[:], ident[:, :])
        qsl = wpool.tile([D, H * QC], BF16, name="qsl")
        # O^T accumulator, all heads side by side: [d, h*QC] fp16
        ot_sb = wpool.tile([D, H * QC], FP16, name="ot_sb")
        if use_bias:
            bqk_t = wpool.tile([D, 2 * H], F32, name="bqk_t")
            nc.sync.dma_start(
                bqk_t.rearrange("d (c h) -> d c h", c=2),
                bqk.rearrange("c h d -> d c h"),
            )
            ones_t = wpool.tile([1, 128], BF16, name="ones_t")
            nc.vector.memset(ones_t[:], 1.0)
            bv_t = wpool.tile([1, E], BF16, name="bv_t")
            nc.sync.dma_start(bv_t[:], bv[:, :])

        # shared PSUM pool: two [128,2048] slots = all 8 banks
        ppsum = ctx.enter_context(
            tc.tile_pool(name="ppsum", bufs=2, space="PSUM"))

        def pslot():
            return ppsum.tile([128, 2048], F32, name="ps", tag="ps")

        # ---- phase 0: Q projection (2 heads per slot) ----
        with ExitStack() as actx:
            qppool = actx.enter_context(tc.tile_pool(name="qppool", bufs=1))
            xqs = []
            for e in range(NE):
                xq_t = qppool.tile([128, QC], BF16, name=f"xq{e}")
                nc.sync.dma_start(xq_t[:], xtq[e * 128:(e + 1) * 128, :])
                xqs.append(xq_t)
            wqs = []
            for e in range(NE):
                wq_t = qppool.tile([128, E], BF16, name=f"wq{e}")
                nc.sync.dma_start(wq_t[:], wq[e * 128:(e + 1) * 128, :])
                wqs.append(wq_t)
            for hp in range(H // 2):
                qp = pslot()
                for hh in range(2):
                    h = 2 * hp + hh
                    for i in range(2):
                        for e in range(NE):
                            nc.tensor.matmul(
                                qp[0:D, hh * QC + i * 512:
                                   hh * QC + (i + 1) * 512],
                                wqs[e][:, h * D:(h + 1) * D],
                                xqs[e][:, i * 512:(i + 1) * 512],
                                start=(e == 0), stop=(e == NE - 1),
                            )
                if use_bias:
                    for hh in range(2):
                        h = 2 * hp + hh
                        nc.scalar.activation(
                            qsl[:, h * QC:(h + 1) * QC],
                            qp[0:D, hh * QC:(hh + 1) * QC],
                            IDENT, bias=bqk_t[:, h:h + 1],
                        )
                else:
                    nc.scalar.copy(
                        qsl[:, 2 * hp * QC:(2 * hp + 2) * QC], qp[0:D, :])

        if DEBUG:
            nc.sync.dma_start(dbg_qsl[:, :], qsl[:])

        # ---- main fused loop over k-blocks ----
        with ExitStack() as bctx:
            xpool = bctx.enter_context(tc.tile_pool(name="xpool", bufs=1))
            ktpool = bctx.enter_context(tc.tile_pool(name="ktpool", bufs=1))
            vtpool = bctx.enter_context(tc.tile_pool(name="vtpool", bufs=1))
            expool = bctx.enter_context(tc.tile_pool(name="expool", bufs=1))
            atpool = bctx.enter_context(tc.tile_pool(name="atpool", bufs=1))
            spool = bctx.enter_context(tc.tile_pool(name="spool", bufs=1))

            def project_kv(kb):
                """K^T and V projections for k-block kb (-> kt/vt tiles)."""
                xks = []
                for e in range(NE):
                    xk_t = xpool.tile([128, KB], BF16, name="xk",
                                      tag="xk", bufs=NE)
                    nc.sync.dma_start(
                        xk_t[:],
                        xt[e * 128:(e + 1) * 128, kb * KB:(kb + 1) * KB])
                    xks.append(xk_t)
                # K^T: 4 heads per psum slot
                ktt = ktpool.tile([D, H * KB], BF16, name="ktt", tag="ktt",
                                  bufs=2)
                for hq in range(2):
                    mp = pslot()
                    for hh in range(4):
                        h = 4 * hq + hh
                        for e in range(NE):
                            nc.tensor.matmul(
                                mp[0:D, hh * KB:(hh + 1) * KB],
                                wks[e][:, h * D:(h + 1) * D],
                                xks[e][:],
                                start=(e == 0), stop=(e == NE - 1),
                            )
                    if use_bias:
                        for hh in range(4):
                            h = 4 * hq + hh
                            nc.scalar.activation(
                                ktt[:, h * KB:(h + 1) * KB],
                                mp[0:D, hh * KB:(hh + 1) * KB],
                                IDENT, bias=bqk_t[:, H + h:H + h + 1],
                            )
                    else:
                        nc.scalar.copy(
                            ktt[:, 4 * hq * KB:(4 * hq + 4) * KB], mp[0:D, :])
                # V: 2 k-chunks per psum slot, packed [128, 2*768]
                vps = []
                for vp2 in range(2):
                    mp = pslot()
                    for cc in range(2):
                        kc4 = 2 * vp2 + cc
                        c0 = cc * 1024  # bank-aligned base for this chunk
                        for e in range(NE):
                            nc.tensor.matmul(
                                mp[:, c0:c0 + 512],
                                xks[e][:, kc4 * 128:(kc4 + 1) * 128],
                                wvs[e][:, 0:512],
                                start=(e == 0), stop=(e == NE - 1),
                            )
                            nc.tensor.matmul(
                                mp[:, c0 + 512:c0 + E],
                                xks[e][:, kc4 * 128:(kc4 + 1) * 128],
                                wvs[e][:, 512:E],
                                start=(e == 0), stop=(e == NE - 1),
                            )
                        if use_bias:
                            nc.tensor.matmul(
                                mp[:, c0:c0 + 512],
                                ones_t[:, 0:128], bv_t[:, 0:512],
                                start=False, stop=True, skip_group_check=True,
                            )
                            nc.tensor.matmul(
                                mp[:, c0 + 512:c0 + E],
                                ones_t[:, 0:128], bv_t[:, 512:E],
                                start=False, stop=True, skip_group_check=True,
                            )
                    v_t = vtpool.tile([128, 2 * E], BF16, name="vt",
                                      tag="vt", bufs=4)
                    nc.vector.tensor_copy(v_t[:, 0:E], mp[:, 0:E])
                    nc.vector.tensor_copy(v_t[:, E:2 * E], mp[:, 1024:1024 + E])
                    vps.append(v_t)
                return ktt, vps

            ktt, vps = project_kv(0)
            if DEBUG:
                nc.sync.dma_start(dbg_ktt[:, :], ktt[:])
                nc.sync.dma_start(dbg_vt[:, :], vps[0][:])
            for kb in range(NKB):
                atts = []
                for kc4 in range(SEG):
                    # energy + exp, 2 heads per slot
                    exa = expool.tile([128, H * QC], BF16, name="exa",
                                      tag="exa", bufs=2)
                    for hp in range(H // 2):
                        ep = pslot()
                        for hh in range(2):
                            h = 2 * hp + hh
                            for i in range(2):
                                nc.tensor.matmul(
                                    ep[:, hh * QC + i * 512:
                                       hh * QC + (i + 1) * 512],
                                    ktt[:, h * KB + kc4 * 128:
                                        h * KB + (kc4 + 1) * 128],
                                    qsl[:, h * QC + i * 512:
                                        h * QC + (i + 1) * 512],
                                    start=True, stop=True,
                                )
                        nc.scalar.activation(
                            exa[:, 2 * hp * QC:(2 * hp + 2) * QC], ep[:], EXPF)
                    # head-sum: wide bf16 tree on DVE
                    t1 = spool.tile([128, 4 * QC], BF16, name="t1", tag="t1",
                                    bufs=1)
                    nc.vector.tensor_add(
                        t1[:], exa[:, 0:4 * QC], exa[:, 4 * QC:8 * QC])
                    t2 = spool.tile([128, 2 * QC], BF16, name="t2", tag="t2",
                                    bufs=1)
                    nc.vector.tensor_add(
                        t2[:], t1[:, 0:2 * QC], t1[:, 2 * QC:4 * QC])
                    s32 = spool.tile([128, QC], F32, name="s32", tag="s32",
                                     bufs=1)
                    nc.vector.tensor_add(s32[:], t2[:, 0:QC], t2[:, QC:2 * QC])
                    r32 = spool.tile([128, QC], F32, name="r32", tag="r32",
                                     bufs=1)
                    nc.vector.reciprocal_approx_fast(r32[:], s32[:])
                    r16 = spool.tile([128, QC], BF16, name="r16", tag="r16",
                                     bufs=2)
                    nc.vector.tensor_scalar_min(r16[:], r32[:], 3e38)
                    # normalize out-of-place -> att tile (DVE 2x mode)
                    att = atpool.tile([128, H * QC], BF16, name="att",
                                      tag="att", bufs=SEG)
                    for h in range(H):
                        nc.vector.tensor_mul(
                            att[:, h * QC:(h + 1) * QC],
                            exa[:, h * QC:(h + 1) * QC],
                            r16[:],
                        )
                    if DEBUG and kb == 0 and kc4 == 0:
                        nc.sync.dma_start(dbg_exa[:, :], exa[:])
                        nc.sync.dma_start(dbg_att[:, :], att[:])
                    atts.append(att)

                # prefetch next block's K/V before Ot to hide softmax latency
                if kb + 1 < NKB:
                    ktt_n, vps_n = project_kv(kb + 1)
                else:
                    ktt_n, vps_n = None, None

                # Ot: 2 heads per slot, psum seeded with running total
                for hp in range(H // 2):
                    op = pslot()
                    for hh in range(2):
                        h = 2 * hp + hh
                        for i in range(2):
                            cols = slice(hh * QC + i * 512,
                                         hh * QC + (i + 1) * 512)
                            qcols = slice(h * QC + i * 512,
                                          h * QC + (i + 1) * 512)
                            if kb > 0:
                                nc.tensor.matmul(
                                    op[0:D, cols],
                                    id_t[0:D, 0:D],
                                    ot_sb[:, qcols],
                                    start=True, stop=False,
                                )
                            for kc4 in range(SEG):
                                nc.tensor.matmul(
                                    op[0:D, cols],
                                    vps[kc4 // 2][:, (kc4 % 2) * E + h * D:
                                                  (kc4 % 2) * E + (h + 1) * D],
                                    atts[kc4][:, qcols],
                                    start=(kb == 0 and kc4 == 0),
                                    stop=(kc4 == SEG - 1),
                                )
                    nc.scalar.copy(
                        ot_sb[:, 2 * hp * QC:(2 * hp + 2) * QC], op[0:D, :])
                ktt, vps = ktt_n, vps_n

        if DEBUG:
            nc.sync.dma_start(dbg_ot[:, :], ot_sb[:])

        # ---- output projection ----
        with ExitStack() as cctx:
            ppool = cctx.enter_context(tc.tile_pool(name="ppool", bufs=1))
            ostp = cctx.enter_context(tc.tile_pool(name="ostp", bufs=2))
            pws = []
            for h in range(H):
                pw_t = ppool.tile([D, E], FP16, name=f"pw{h}")
                nc.sync.dma_start(pw_t[:], pw[h * D:(h + 1) * D, :])
                pws.append(pw_t)
            for qb in range(QC // 128):
                po = pslot()
                for h in range(H):
                    lhs = ot_sb[:, h * QC + qb * 128:h * QC + (qb + 1) * 128]
                    nc.tensor.matmul(
                        po[:, 0:512], lhs, pws[h][:, 0:512],
                        start=(h == 0), stop=(h == H - 1))
                    nc.tensor.matmul(
                        po[:, 512:E], lhs, pws[h][:, 512:E],
                        start=(h == 0), stop=(h == H - 1))
                ost = ostp.tile([128, E], F32, name="ost")
                nc.scalar.copy(ost[:], po[:, 0:E])
                nc.sync.dma_start(out[qb * 128:(qb + 1) * 128, :], ost[:])

    nc.compile()
    return nc


_CACHE = {}


def _get_program(use_bias: bool):
    if use_bias not in _CACHE:
        _CACHE[use_bias] = build(use_bias)
    return _CACHE[use_bias]


def _prep_inputs(x, qkv_w, qkv_b, proj_w):
    bf = ml_dtypes.bfloat16
    qw = np.ascontiguousarray(qkv_w.reshape(E, H, D, 3))
    wq = np.ascontiguousarray(qw[..., 0].reshape(E, E)).astype(bf)
    wk = np.ascontiguousarray(qw[..., 1].reshape(E, E)).astype(bf)
    wv = np.ascontiguousarray(qw[..., 2].reshape(E, E)).astype(bf)
    pw = np.ascontiguousarray(
        proj_w / np.sqrt(np.float32(E))).astype(np.float16)
    qb = qkv_b.reshape(H, D, 3)
    bqk = np.ascontiguousarray(
        np.stack([qb[..., 0], qb[..., 1]], axis=0)).astype(np.float32)
    bv = np.ascontiguousarray(qb[..., 2].reshape(1, E)).astype(bf)
    xts = [np.ascontiguousarray(x[b].T).astype(bf) for b in range(B)]
    in_maps = []
    for c in range(NCORES):
        b, qi = c // 4, c % 4
        in_maps.append({
            "xt": xts[b],
            "xtq": np.ascontiguousarray(xts[b][:, qi * QC:(qi + 1) * QC]),
            "wq": wq, "wk": wk, "wv": wv, "pw": pw,
            "bqk": bqk, "bv": bv,
            "ident": np.eye(128, dtype=np.float16),
        })
    return in_maps


def kernel(x, qkv_w, qkv_b, proj_w, proj_b, _trace=False):
    x = np.asarray(x, dtype=np.float32)
    qkv_w = np.asarray(qkv_w, dtype=np.float32)
    qkv_b = np.asarray(qkv_b, dtype=np.float32)
    proj_w = np.asarray(proj_w, dtype=np.float32)
    proj_b = np.asarray(proj_b, dtype=np.float32)

    use_bias = bool(np.any(qkv_b))
    nc = _get_program(use_bias)
    in_maps = _prep_inputs(x, qkv_w, qkv_b, proj_w)
    res = bass_utils.run_bass_kernel_spmd(
        nc, in_maps, core_ids=list(range(NCORES)), trace=_trace)
    outf = np.empty((B, N, E), dtype=np.float32)
    for c in range(NCORES):
        b, qi = c // 4, c % 4
        outf[b, qi * QC:(qi + 1) * QC, :] = res.results[c]["out"]
    if np.any(proj_b):
        outf += proj_b[None, None, :]
    if _trace:
        kernel.last_exec_time_ns = res.exec_time_ns
        kernel.last_results = res
    return outf


# revision 11
# speedup vs baseline: 1.4631x; 1.1424x over previous
"""Trainium2 Bass kernel for nn_MultiHeadAttention_17386027615012.

MHA variant: softmax over the HEAD axis (dim=1), 1/sqrt(emb) applied AFTER
softmax (folded into proj_w host-side). Softmax-over-heads makes every (q, k)
independent -> shard flattened (batch, seq) query rows over 8 cores
(b = core // 4, 1024-row q-chunk = core % 4), zero inter-core comms; each
core computes its batch's full K/V (4x redundant).

v2.1: fused single-pass streaming, SBUF-resident (no DRAM staging).
All PSUM work shares one pool of two [128,2048] slots (8 banks):
 - K proj packs 4 heads/slot, V proj 2 k-chunks/slot, energy 2 heads/slot
   (-> one wide [128,2048] exp ACTIVATE per head-pair), Ot 2 heads/slot.
 - Ot accumulation across 4-kc segments seeds each new PSUM group with the
   running total via an identity matmul (fp16), so cross-segment adds cost
   PE columns instead of DVE passes; drains are wide ACT copies into a
   single fp16 ot_sb tile that the output projection consumes directly.
Softmax: wide DVE bf16 tree sum (4096/2048/1024), fast reciprocal, clamp on
GpSimd, out-of-place bf16 normalize muls on DVE (2x mode).
K/V projections for block kb+1 issue before Ot(kb) to hide softmax latency.
"""
import sys

sys.path.insert(0, "/opt/trn_rl_repo")

import numpy as np
import ml_dtypes
from contextlib import ExitStack

import concourse.bass as bass
import concourse.tile as tile
from concourse import bacc, mybir
from concourse import bass_utils

F32 = mybir.dt.float32
BF16 = mybir.dt.bfloat16
FP16 = mybir.dt.float16
EXPF = mybir.ActivationFunctionType.Exp
IDENT = mybir.ActivationFunctionType.Identity

B, N, E, H, D = 2, 4096, 768, 8, 96
NCORES = 4 * B
QC = N // 4          # 1024 q rows per core
KB = 512             # k-block (projection granularity)
NKB = N // KB        # 8
SEG = 4              # k-chunks per Ot psum segment (= one k-block)
NE = E // 128        # 6


DEBUG = False


def build(use_bias: bool):
    nc = bacc.Bacc("TRN2", debug=False)
    xt = nc.dram_tensor("xt", (E, N), BF16, kind="ExternalInput").ap()
    xtq = nc.dram_tensor("xtq", (E, QC), BF16, kind="ExternalInput").ap()
    wq = nc.dram_tensor("wq", (E, E), BF16, kind="ExternalInput").ap()
    wk = nc.dram_tensor("wk", (E, E), BF16, kind="ExternalInput").ap()
    wv = nc.dram_tensor("wv", (E, E), BF16, kind="ExternalInput").ap()
    pw = nc.dram_tensor("pw", (E, E), FP16, kind="ExternalInput").ap()
    bqk = nc.dram_tensor("bqk", (2, H, D), F32, kind="ExternalInput").ap()
    bv = nc.dram_tensor("bv", (1, E), BF16, kind="ExternalInput").ap()
    ident = nc.dram_tensor("ident", (128, 128), FP16, kind="ExternalInput").ap()
    out = nc.dram_tensor("out", (QC, E), F32, kind="ExternalOutput").ap()
    if DEBUG:
        dbg_qsl = nc.dram_tensor("dbg_qsl", (D, H * QC), BF16,
                                 kind="ExternalOutput").ap()
        dbg_ktt = nc.dram_tensor("dbg_ktt", (D, H * KB), BF16,
                                 kind="ExternalOutput").ap()
        dbg_vt = nc.dram_tensor("dbg_vt", (128, 2 * E), BF16,
                                kind="ExternalOutput").ap()
        dbg_exa = nc.dram_tensor("dbg_exa", (128, H * QC), BF16,
                                 kind="ExternalOutput").ap()
        dbg_att = nc.dram_tensor("dbg_att", (128, H * QC), BF16,
                                 kind="ExternalOutput").ap()
        dbg_ot = nc.dram_tensor("dbg_ot", (D, H * QC), FP16,
                                kind="ExternalOutput").ap()

    with tile.TileContext(nc) as tc, ExitStack() as ctx:
        # ---- persistent pools ----
        wpool = ctx.enter_context(tc.tile_pool(name="wpool", bufs=1))
        wks, wvs = [], []
        for e in range(NE):
            wk_t = wpool.tile([128, E], BF16, name=f"wk{e}")
            nc.sync.dma_start(wk_t[:], wk[e * 128:(e + 1) * 128, :])
            wks.append(wk_t)
        for e in range(NE):
            wv_t = wpool.tile([128, E], BF16, name=f"wv{e}")
            nc.sync.dma_start(wv_t[:], wv[e * 128:(e + 1) * 128, :])
            wvs.append(wv_t)
        id_t = wpool.tile([128, 128], FP16, name="id_t")
        nc.sync.dma_start(id_t[:], ident[:, :])
        qsl = wpool.tile([D, H * QC], BF16, name="qsl")
        # O^T accumulator, all heads side by side: [d, h*QC] fp16
        ot_sb = wpool.tile([D, H * QC], FP16, name="ot_sb")
        if use_bias:
            bqk_t = wpool.tile([D, 2 * H], F32, name="bqk_t")
            nc.sync.dma_start(
                bqk_t.rearrange("d (c h) -> d c h", c=2),
                bqk.rearrange("c h d -> d c h"),
            )
            ones_t = wpool.tile([1, 128], BF16, name="ones_t")
            nc.vector.memset(ones_t[:], 1.0)
            bv_t = wpool.tile([1, E], BF16, name="bv_t")
            nc.sync.dma_start(bv_t[:], bv[:, :])

        # shared PSUM pool: two [128,2048] slots = all 8 banks
        ppsum = ctx.enter_context(
            tc.tile_pool(name="ppsum", bufs=2, space="PSUM"))

        def pslot():
            return ppsum.tile([128, 2048], F32, name="ps", tag="ps")

        # ---- phase 0: Q projection (2 heads per slot) ----
        with ExitStack() as actx:
            qppool = actx.enter_context(tc.tile_pool(name="qppool", bufs=1))
            xqs = []
            for e in range(NE):
                xq_t = qppool.tile([128, QC], BF16, name=f"xq{e}")
                nc.sync.dma_start(xq_t[:], xtq[e * 128:(e + 1) * 128, :])
                xqs.append(xq_t)
            wqs = []
            for e in range(NE):
                wq_t = qppool.tile([128, E], BF16, name=f"wq{e}")
                nc.sync.dma_start(wq_t[:], wq[e * 128:(e + 1) * 128, :])
                wqs.append(wq_t)
            for hp in range(H // 2):
                qp = pslot()
                for hh in range(2):
                    h = 2 * hp + hh
                    for i in range(2):
                        for e in range(NE):
                            nc.tensor.matmul(
                                qp[0:D, hh * QC + i * 512:
                                   hh * QC + (i + 1) * 512],
                                wqs[e][:, h * D:(h + 1) * D],
                                xqs[e][:, i * 512:(i + 1) * 512],
                                start=(e == 0), stop=(e == NE - 1),
                            )
                if use_bias:
                    for hh in range(2):
                        h = 2 * hp + hh
                        nc.scalar.activation(
                            qsl[:, h * QC:(h + 1) * QC],
                            qp[0:D, hh * QC:(hh + 1) * QC],
                            IDENT, bias=bqk_t[:, h:h + 1],
                        )
                else:
                    nc.scalar.copy(
                        qsl[:, 2 * hp * QC:(2 * hp + 2) * QC], qp[0:D, :])

        if DEBUG:
            nc.sync.dma_start(dbg_qsl[:, :], qsl[:])

        # ---- main fused loop over k-blocks ----
        with ExitStack() as bctx:
            xpool = bctx.enter_context(tc.tile_pool(name="xpool", bufs=1))
            ktpool = bctx.enter_context(tc.tile_pool(name="ktpool", bufs=1))
            vtpool = bctx.enter_context(tc.tile_pool(name="vtpool", bufs=1))
            expool = bctx.enter_context(tc.tile_pool(name="expool", bufs=1))
            atpool = bctx.enter_context(tc.tile_pool(name="atpool", bufs=1))
            spool = bctx.enter_context(tc.tile_pool(name="spool", bufs=1))

            def dma_x(kb):
                xks = []
                for e in range(NE):
                    xk_t = xpool.tile([128, KB], BF16, name="xk",
                                      tag="xk", bufs=NE + 2)
                    nc.sync.dma_start(
                        xk_t[:],
                        xt[e * 128:(e + 1) * 128, kb * KB:(kb + 1) * KB])
                    xks.append(xk_t)
                return xks

            def kproj_unit(xks, ktt, hq):
                mp = pslot()
                for hh in range(4):
                    h = 4 * hq + hh
                    for e in range(NE):
                        nc.tensor.matmul(
                            mp[0:D, hh * KB:(hh + 1) * KB],
                            wks[e][:, h * D:(h + 1) * D],
                            xks[e][:],
                            start=(e == 0), stop=(e == NE - 1),
                        )
                if use_bias:
                    for hh in range(4):
                        h = 4 * hq + hh
                        nc.scalar.activation(
                            ktt[:, h * KB:(h + 1) * KB],
                            mp[0:D, hh * KB:(hh + 1) * KB],
                            IDENT, bias=bqk_t[:, H + h:H + h + 1],
                        )
                else:
                    nc.scalar.copy(
                        ktt[:, 4 * hq * KB:(4 * hq + 4) * KB], mp[0:D, :])

            def vproj_unit(xks, vp2):
                mp = pslot()
                for cc in range(2):
                    kc4 = 2 * vp2 + cc
                    c0 = cc * 1024  # bank-aligned base for this chunk
                    for e in range(NE):
                        nc.tensor.matmul(
                            mp[:, c0:c0 + 512],
                            xks[e][:, kc4 * 128:(kc4 + 1) * 128],
                            wvs[e][:, 0:512],
                            start=(e == 0), stop=(e == NE - 1),
                        )
                        nc.tensor.matmul(
                            mp[:, c0 + 512:c0 + E],
                            xks[e][:, kc4 * 128:(kc4 + 1) * 128],
                            wvs[e][:, 512:E],
                            start=(e == 0), stop=(e == NE - 1),
                        )
                    if use_bias:
                        nc.tensor.matmul(
                            mp[:, c0:c0 + 512],
                            ones_t[:, 0:128], bv_t[:, 0:512],
                            start=False, stop=True, skip_group_check=True,
                        )
                        nc.tensor.matmul(
                            mp[:, c0 + 512:c0 + E],
                            ones_t[:, 0:128], bv_t[:, 512:E],
                            start=False, stop=True, skip_group_check=True,
                        )
                v_t = vtpool.tile([128, 2 * E], BF16, name="vt",
                                  tag="vt", bufs=4)
                nc.vector.tensor_copy(v_t[:, 0:E], mp[:, 0:E])
                nc.vector.tensor_copy(v_t[:, E:2 * E], mp[:, 1024:1024 + E])
                return v_t

            def energy_unit(ktt, kc4, hp, exa):
                ep = pslot()
                for hh in range(2):
                    h = 2 * hp + hh
                    for i in range(2):
                        nc.tensor.matmul(
                            ep[:, hh * QC + i * 512:
                               hh * QC + (i + 1) * 512],
                            ktt[:, h * KB + kc4 * 128:
                                h * KB + (kc4 + 1) * 128],
                            qsl[:, h * QC + i * 512:
                                h * QC + (i + 1) * 512],
                            start=True, stop=True,
                        )
                nc.scalar.activation(
                    exa[:, 2 * hp * QC:(2 * hp + 2) * QC], ep[:], EXPF)

            def softmax_unit(exa):
                ta = spool.tile([128, 2 * QC], BF16, name="ta", tag="ta",
                                bufs=1)
                nc.vector.tensor_add(
                    ta[:], exa[:, 0:2 * QC], exa[:, 2 * QC:4 * QC])
                tb = spool.tile([128, 2 * QC], BF16, name="tb", tag="tb",
                                bufs=1)
                nc.vector.tensor_add(
                    tb[:], exa[:, 4 * QC:6 * QC], exa[:, 6 * QC:8 * QC])
                tc = spool.tile([128, 2 * QC], BF16, name="tc", tag="tc",
                                bufs=1)
                nc.vector.tensor_add(tc[:], ta[:], tb[:])
                s32 = spool.tile([128, QC], F32, name="s32", tag="s32",
                                 bufs=1)
                nc.vector.tensor_add(s32[:], tc[:, 0:QC], tc[:, QC:2 * QC])
                r32 = spool.tile([128, QC], F32, name="r32", tag="r32",
                                 bufs=1)
                nc.vector.reciprocal_approx_fast(r32[:], s32[:])
                r16 = spool.tile([128, QC], BF16, name="r16", tag="r16",
                                 bufs=2)
                nc.vector.tensor_scalar_min(r16[:], r32[:], 3e38)
                att = atpool.tile([128, H * QC], BF16, name="att",
                                  tag="att", bufs=SEG)
                nc.vector.tensor_mul(
                    att[:].rearrange("p (h q) -> p h q", h=H),
                    exa[:].rearrange("p (h q) -> p h q", h=H),
                    r16[:, None, :].to_broadcast((128, H, QC)),
                )
                return att

            def ot_unit(kb_prev, vps, atts, hp):
                op = pslot()
                for hh in range(2):
                    h = 2 * hp + hh
                    for i in range(2):
                        cols = slice(hh * QC + i * 512,
                                     hh * QC + (i + 1) * 512)
                        qcols = slice(h * QC + i * 512,
                                      h * QC + (i + 1) * 512)
                        if kb_prev > 0:
                            nc.tensor.matmul(
                                op[0:D, cols],
                                id_t[0:D, 0:D],
                                ot_sb[:, qcols],
                                start=True, stop=False,
                            )
                        for kc4 in range(SEG):
                            nc.tensor.matmul(
                                op[0:D, cols],
                                vps[kc4 // 2][:, (kc4 % 2) * E + h * D:
                                              (kc4 % 2) * E + (h + 1) * D],
                                atts[kc4][:, qcols],
                                start=(kb_prev == 0 and kc4 == 0),
                                stop=(kc4 == SEG - 1),
                            )
                nc.scalar.copy(
                    ot_sb[:, 2 * hp * QC:(2 * hp + 2) * QC], op[0:D, :])

            # ---- software-pipelined block schedule ----
            # steady state for block kb:
            #   energy/softmax(kb) interleaved with Ot(kb-1) and K/V proj(kb+1)
            xks = dma_x(0)
            ktt = ktpool.tile([D, H * KB], BF16, name="ktt", tag="ktt",
                              bufs=2)
            kproj_unit(xks, ktt, 0)
            kproj_unit(xks, ktt, 1)
            vps = [vproj_unit(xks, 0), vproj_unit(xks, 1)]
            prev = None  # (kb, vps, atts) awaiting Ot
            for kb in range(NKB):
                if kb + 1 < NKB:
                    xks_n = dma_x(kb + 1)
                    ktt_n = ktpool.tile([D, H * KB], BF16, name="ktt",
                                        tag="ktt", bufs=2)
                else:
                    xks_n, ktt_n = None, None
                vps_n = []
                atts = []
                # X units woven between energy half-kcs:
                # [Ot(prev) x4, KP(kb+1) x2, VP(kb+1) x2]
                def xunit(j):
                    if j < 4:
                        if prev is not None:
                            ot_unit(prev[0], prev[1], prev[2], j)
                    elif j < 6:
                        if ktt_n is not None:
                            kproj_unit(xks_n, ktt_n, j - 4)
                    else:
                        if ktt_n is not None:
                            vps_n.append(vproj_unit(xks_n, j - 6))
                for kc4 in range(SEG):
                    exa = expool.tile([128, H * QC], BF16, name="exa",
                                      tag="exa", bufs=2)
                    energy_unit(ktt, kc4, 0, exa)
                    energy_unit(ktt, kc4, 1, exa)
                    xunit(2 * kc4)
                    energy_unit(ktt, kc4, 2, exa)
                    energy_unit(ktt, kc4, 3, exa)
                    atts.append(softmax_unit(exa))
                    xunit(2 * kc4 + 1)
                prev = (kb, vps, atts)
                ktt, vps, xks = ktt_n, vps_n, xks_n
            # trailing Ot for the last block
            for hp in range(4):
                ot_unit(prev[0], prev[1], prev[2], hp)

        # ---- output projection ----
        with ExitStack() as cctx:
            ppool = cctx.enter_context(tc.tile_pool(name="ppool", bufs=1))
            ostp = cctx.enter_context(tc.tile_pool(name="ostp", bufs=2))
            pws = []
            for h in range(H):
                pw_t = ppool.tile([D, E], FP16, name=f"pw{h}")
                nc.sync.dma_start(pw_t[:], pw[h * D:(h + 1) * D, :])
                pws.append(pw_t)
            for qb in range(QC // 128):
                po = pslot()
                for h in range(H):
                    lhs = ot_sb[:, h * QC + qb * 128:h * QC + (qb + 1) * 128]
                    nc.tensor.matmul(
                        po[:, 0:512], lhs, pws[h][:, 0:512],
                        start=(h == 0), stop=(h == H - 1))
                    nc.tensor.matmul(
                        po[:, 512:E], lhs, pws[h][:, 512:E],
                        start=(h == 0), stop=(h == H - 1))
                ost = ostp.tile([128, E], F32, name="ost")
                nc.scalar.copy(ost[:], po[:, 0:E])
                nc.sync.dma_start(out[qb * 128:(qb + 1) * 128, :], ost[:])

    nc.compile()
    return nc


_CACHE = {}


def _get_program(use_bias: bool):
    if use_bias not in _CACHE:
        _CACHE[use_bias] = build(use_bias)
    return _CACHE[use_bias]


def _prep_inputs(x, qkv_w, qkv_b, proj_w):
    bf = ml_dtypes.bfloat16
    qw = np.ascontiguousarray(qkv_w.reshape(E, H, D, 3))
    wq = np.ascontiguousarray(qw[..., 0].reshape(E, E)).astype(bf)
    wk = np.ascontiguousarray(qw[..., 1].reshape(E, E)).astype(bf)
    wv = np.ascontiguousarray(qw[..., 2].reshape(E, E)).astype(bf)
    pw = np.ascontiguousarray(
        proj_w / np.sqrt(np.float32(E))).astype(np.float16)
    qb = qkv_b.reshape(H, D, 3)
    bqk = np.ascontiguousarray(
        np.stack([qb[..., 0], qb[..., 1]], axis=0)).astype(np.float32)
    bv = np.ascontiguousarray(qb[..., 2].reshape(1, E)).astype(bf)
    xts = [np.ascontiguousarray(x[b].T).astype(bf) for b in range(B)]
    in_maps = []
    for c in range(NCORES):
        b, qi = c // 4, c % 4
        in_maps.append({
            "xt": xts[b],
            "xtq": np.ascontiguousarray(xts[b][:, qi * QC:(qi + 1) * QC]),
            "wq": wq, "wk": wk, "wv": wv, "pw": pw,
            "bqk": bqk, "bv": bv,
            "ident": np.eye(128, dtype=np.float16),
        })
    return in_maps


def kernel(x, qkv_w, qkv_b, proj_w, proj_b, _trace=False):
    x = np.asarray(x, dtype=np.float32)
    qkv_w = np.asarray(qkv_w, dtype=np.float32)
    qkv_b = np.asarray(qkv_b, dtype=np.float32)
    proj_w = np.asarray(proj_w, dtype=np.float32)
    proj_b = np.asarray(proj_b, dtype=np.float32)

    use_bias = bool(np.any(qkv_b))
    nc = _get_program(use_bias)
    in_maps = _prep_inputs(x, qkv_w, qkv_b, proj_w)
    res = bass_utils.run_bass_kernel_spmd(
        nc, in_maps, core_ids=list(range(NCORES)), trace=_trace)
    outf = np.empty((B, N, E), dtype=np.float32)
    for c in range(NCORES):
        b, qi = c // 4, c % 4
        outf[b, qi * QC:(qi + 1) * QC, :] = res.results[c]["out"]
    if np.any(proj_b):
        outf += proj_b[None, None, :]
    if _trace:
        kernel.last_exec_time_ns = res.exec_time_ns
        kernel.last_results = res
    return outf
